# revision 52
# baseline (speedup 1.0000x reference)
"""Trainium2 Bass kernel for GNN message passing (nn_MessagePassing).

Reference computation (N=5000 nodes, E=40000 edges, U=64, EF=16, 4 steps):
    edge_mats = (edge_features @ edge_kernel + edge_bias).reshape(E, 64, 64)
    h = node_features
    4x:  nb  = h[nbr]
         msg = einsum('eij,ej->ei', edge_mats, nb)
         agg = segment_sum(msg, src, N)
         h   = GRU(agg, h)        # Keras GRUCell, reset_after=True

Device algorithm (avoids materializing the 655MB edge_mats):
  Because edge_mats_e = sum_f ef[e,f] * W_f (+ bias mat), the per-step compute
  factorizes into dense matmuls around a *static* scaled-scatter matrix S:
    stage 1:  u[j, f, n] = sum_{e: src(e)=n} S_f[e] * h[nbr_e, j]
              (per 128-edge bin: one PE matmul lhsT=gathered nb [128,64],
               rhs=S bin [128, 17*16]; one bin == one chunk, cpb=1)
    stage 2:  agg^T[i, n] = sum_f W2_f^T @ u_f     (W2 = relaid edge_kernel,
              17th channel carries edge_bias with S_16 = 1)
    GRU     : feature-major matmuls + DVE/ACT elementwise, own 640 nodes only
    exchange: PE-transpose own h slice (fp16) -> AllGather fp16 (steps 0..2)
  Edges are sharded by destination node across 8 cores; nodes are assigned to
  cores/bins by a balanced partition (host-side index relayout only, no math).
  Bins hold WBIN=16 nodes and at most 128 edges (verified at preprocessing),
  so each bin is exactly one 128-row gather chunk: stage-1 PE time is half of
  the 28-node/2-chunk layout, and the gather is 5120 descriptors/core.

The h state is kept in fp16 (stage-1 consumed h as fp16 anyway); the
exchange stays fp32 because dma_gather descriptors must move >=256B per
index (one 64-wide fp32 row); the pack transposes upcast f16 h on the fly.

Gather pipelining: per-edge dma_gather split into descriptor PREP (gpsimd
SWDGE desc-gen, runs during the previous step) and TRIGGER (fires right after
the AllGather lands).  4 pieces per step so stage-1 matmuls on early pieces
overlap later pieces' DMA.  True DMA completion is user-managed via gsems
(+16 on piece done); the per-piece f32->f16 CAST carries the gsem wait and
gates that piece's stage-1 LDWEIGHTS.  Preps ride SWDGE queues 1-3 (queue 0
carries Tile's IncSwdgeSem pre-bumps whose doorbells would fire co-queued
descriptors).
"""

import os
import sys
import time

import numpy as np

sys.path.insert(0, "/opt/trn_rl_repo")

# ---------------------------------------------------------------- constants
N, E, U, EF, STEPS, NCORES = 5000, 40000, 64, 16, 4, 8
NPC = 640                    # nodes per core (5120 = 8*640 padded)
NPAD = NPC * NCORES
WBIN = 16                    # node slots per bin
NBINS = 40                   # bins per core -> 40*16 = 640 slots (== NPC)
SLOTS = WBIN * NBINS
FCH = EF + 1                 # 16 edge-feature channels + 1 bias channel
SW = FCH * WBIN              # S chunk width (272)
P = 128
NPAIR = (FCH + 1) // 2       # stage-2 channel pairs on 128 partitions (9)
NH0 = 384                    # GRU/stage-2 half widths (3*128 / 2*128)
NH1 = SLOTS - NH0
NBH0 = NH0 // WBIN           # bins covering half 0 (24)
# Gather-piece bin boundaries. Piece 1 ends exactly at the half-0 boundary
# (bin 24) so stage-2/GRU of half 0 can start; piece 0 small so stage-1
# starts early (the first ~1.5k descriptors pace ~7ns each while remote
# cores' AllGather tail still shares the DMA fabric; later ones ~1ns).
PBINS = [0, 8, 24, 32, NBINS]
# Global h layout is half-major so each half's AllGather lands contiguously:
# rows [0, 8*384) hold every core's half-0 columns, rows [8*384, 8*640) the
# half-1 columns.
NROWS_A = NCORES * NH0

_cache = {}


# ------------------------------------------------------------ preprocessing
def _preprocess(node_features, edge_features, pair_indices):
    """Pure index/relayout work (no float arithmetic on tensor values)."""
    src = np.asarray(pair_indices[:, 0], dtype=np.int64)
    nbr = np.asarray(pair_indices[:, 1], dtype=np.int64)
    ef = np.asarray(edge_features, dtype=np.float32)
    nf = np.asarray(node_features, dtype=np.float32)

    deg = np.bincount(src, minlength=N)

    # 1) assign nodes to cores, balancing edge counts (greedy by degree desc)
    order = np.argsort(-deg, kind="stable")
    core_load = np.zeros(NCORES, dtype=np.int64)
    core_cnt = np.zeros(NCORES, dtype=np.int64)
    node_core = np.empty(N, dtype=np.int64)
    for n in order:
        c = -1
        best = None
        for k in range(NCORES):
            if core_cnt[k] < NPC and (best is None or core_load[k] < best):
                best = core_load[k]
                c = k
        node_core[n] = c
        core_load[c] += deg[n]
        core_cnt[c] += 1

    # 2) per-core bin packing: NBINS bins of exactly WBIN node slots; balance
    #    edges per bin greedily so every bin holds <= 128 edges (cpb == 1).
    node_col = np.empty(N, dtype=np.int64)      # column within core [0, 640)
    bins_nodes = [[[] for _ in range(NBINS)] for _ in range(NCORES)]
    for c in range(NCORES):
        nodes_c = np.where(node_core == c)[0]
        nodes_c = nodes_c[np.argsort(-deg[nodes_c], kind="stable")]
        bin_load = np.zeros(NBINS, dtype=np.int64)
        bin_cnt = np.zeros(NBINS, dtype=np.int64)
        for n in nodes_c:
            b = -1
            best = None
            for k in range(NBINS):
                if bin_cnt[k] < WBIN and (best is None or bin_load[k] < best):
                    best = bin_load[k]
                    b = k
            bins_nodes[c][b].append(n)
            node_col[n] = b * WBIN + bin_cnt[b]
            bin_load[b] += deg[n]
            bin_cnt[b] += 1

    max_bin_edges = 0
    for c in range(NCORES):
        for b in range(NBINS):
            tot = int(sum(deg[n] for n in bins_nodes[c][b]))
            max_bin_edges = max(max_bin_edges, tot)
    assert max_bin_edges <= P, (
        f"bin overflow: {max_bin_edges} edges > {P}; layout needs cpb>1")

    # 3) build S [core][128, NBINS, SW], gidx [core][128, NBINS] int32
    edges_of_node = [[] for _ in range(N)]
    for e in range(E):
        edges_of_node[src[e]].append(e)

    assert node_col.max() < NPC

    S = np.zeros((NCORES, P, NBINS, SW), dtype=np.float16)
    gidx = np.zeros((NCORES, P, NBINS), dtype=np.int32)
    nbr_id = np.zeros((NCORES, P, NBINS), dtype=np.int64)
    for c in range(NCORES):
        for b in range(NBINS):
            elist = []
            slot_of = {}
            for s_i, n in enumerate(bins_nodes[c][b]):
                slot_of[n] = s_i
                elist.extend(edges_of_node[n])
            assert len(elist) <= P
            for p, e in enumerate(elist):
                s_i = slot_of[src[e]]
                S[c, p, b, np.arange(EF) * WBIN + s_i] = ef[e]
                S[c, p, b, EF * WBIN + s_i] = 1.0
                m = nbr[e]
                nbr_id[c, p, b] = m
                gidx[c, p, b] = node_core[m] * NPC + node_col[m]

    # 4) initial h^T per core [64, SLOTS] fp16
    h0t16 = np.zeros((NCORES, U, SLOTS), dtype=np.float16)
    for n in range(N):
        c = node_core[n]
        h0t16[c, :, node_col[n]] = nf[n].astype(np.float16)

    # Step-0's gather source is a host input, so pre-gather it on the host
    # (pure indexing): nb0[c, p, b, :] = nf16[nbr of edge at (p, b)].
    # This removes step 0's on-device gather AND its ~40us of serial
    # descriptor generation.  Pad slots point at node 0 (defined values,
    # zero S rows).
    nf16 = nf.astype(np.float16)
    nb0 = np.ascontiguousarray(nf16[nbr_id])        # [NCORES, P, NBINS, U]

    # dma_gather int16 index layout: flat index i = chunk*128 + lane,
    # wrapped as idx16[i % 16, i // 16], replicated across the 8
    # 16-partition groups.  One independent wrapped block per gather piece
    # (bin groups PBINS), concatenated along the free dim.
    gidx16 = np.zeros((NCORES, P, NBINS * P // 16), dtype=np.int16)
    for c in range(NCORES):
        off = 0
        for lo, hi in zip(PBINS[:-1], PBINS[1:]):
            flat = gidx[c][:, lo:hi].T.reshape(-1)     # i = k*128 + p
            nidx = flat.size
            ncol = nidx // 16
            wrapped = np.zeros((16, ncol), dtype=np.int16)
            wrapped[np.arange(nidx) % 16, np.arange(nidx) // 16] = flat
            gidx16[c][:, off:off + ncol] = np.tile(wrapped, (P // 16, 1))
            off += ncol
    return dict(S=S, gidx16=gidx16, h0t16=h0t16, nb0=nb0,
                node_core=node_core, node_col=node_col)


def _prep_weights(edge_kernel, edge_bias, gru_kernel, gru_recurrent_kernel,
                  gru_bias):
    ek = np.asarray(edge_kernel, dtype=np.float32).reshape(EF, U, U)
    w2 = np.empty((U, FCH, U), dtype=np.float32)        # [j, f, i]
    w2[:, :EF, :] = np.transpose(ek, (2, 0, 1))         # w2[j,f,i]=ek[f,i,j]
    w2[:, EF, :] = np.asarray(edge_bias, dtype=np.float32).reshape(U, U).T
    w2 = w2.reshape(U, FCH * U)

    # Channel-PAIR layout for stage 2: two 64-row channel blocks stacked on
    # 128 partitions per matmul (full PE contraction), 9 matmuls instead of
    # 17.  Pair 8's upper half is zero (channel 16 has no partner; the
    # device writes defined data there so 0 * x == 0).
    w2p = np.zeros((P, NPAIR * U), dtype=np.float32)
    for q in range(EF // 2):
        w2p[0:U, q * U:(q + 1) * U] = w2[:, (2 * q) * U:(2 * q + 1) * U]
        w2p[U:2 * U, q * U:(q + 1) * U] = w2[:, (2 * q + 1) * U:(2 * q + 2) * U]
    w2p[0:U, (NPAIR - 1) * U:] = w2[:, EF * U:]

    gb = np.asarray(gru_bias, dtype=np.float32)
    gbzr = np.stack([gb[0, 0:U] + gb[1, 0:U],
                     gb[0, U:2 * U] + gb[1, U:2 * U]], axis=1)   # [64, 2]
    gbh0 = gb[0, 2 * U:3 * U].reshape(U, 1)
    gbh1 = gb[1, 2 * U:3 * U].reshape(U, 1)
    return dict(w2=w2p.astype(np.float16),
                gk=np.asarray(gru_kernel, dtype=np.float16),
                grk=np.asarray(gru_recurrent_kernel, dtype=np.float16),
                gbzr=gbzr, gbh0=gbh0, gbh1=gbh1)


# ------------------------------------------------------------- bass program
def _build_program():
    from concourse import bacc, mybir, tile
    import concourse.bass as bass
    from concourse import library_config

    f32 = mybir.dt.float32
    f16 = mybir.dt.float16
    AF = mybir.ActivationFunctionType

    nc = bacc.Bacc("TRN2", target_bir_lowering=False, debug=False,
                   num_devices=NCORES, num_swdge_queues=4)

    # gather pieces: bin(==chunk) ranges + idx-column ranges per piece
    pieces = []                   # (bin_lo, bin_hi, idxcol_lo, idxcol_hi)
    off = 0
    for lo, hi in zip(PBINS[:-1], PBINS[1:]):
        w = (hi - lo) * P // 16
        pieces.append((lo, hi, off, off + w))
        off += w
    NPIECE = len(pieces)
    totcol = NBINS * P // 16

    # ---- I/O
    t_s = nc.dram_tensor("s_mat", [P, NBINS, SW], f16, kind="ExternalInput")
    t_gidx = nc.dram_tensor("gidx16", [P, totcol], mybir.dt.int16, kind="ExternalInput")
    t_h0t16 = nc.dram_tensor("h0t16", [U, SLOTS], f16, kind="ExternalInput")
    t_nb0 = nc.dram_tensor("nb0", [P, NBINS, U], f16, kind="ExternalInput")
    t_ident = nc.dram_tensor("ident", [U, U], f16, kind="ExternalInput")
    t_w2 = nc.dram_tensor("w2", [P, NPAIR * U], f16, kind="ExternalInput")
    t_gk = nc.dram_tensor("gk", [U, 3 * U], f16, kind="ExternalInput")
    t_grk = nc.dram_tensor("grk", [U, 3 * U], f16, kind="ExternalInput")
    t_gbzr = nc.dram_tensor("gbzr", [U, 2], f32, kind="ExternalInput")
    t_gbh0 = nc.dram_tensor("gbh0", [U, 1], f32, kind="ExternalInput")
    t_gbh1 = nc.dram_tensor("gbh1", [U, 1], f32, kind="ExternalInput")
    t_out = nc.dram_tensor("h_out", [U, SLOTS], f16, kind="ExternalOutput")

    with tile.TileContext(nc) as tc:
        with (
            tc.tile_pool(name="const", bufs=1) as cpool,
            tc.tile_pool(name="work", bufs=2) as wpool,
            tc.tile_pool(name="psum", bufs=1, space="PSUM") as pp,
            tc.tile_pool(name="dram", bufs=1, space="DRAM") as dpool,
        ):
            # ---- constants into SBUF (gather index table first: step-0
            # descriptor generation depends only on it + the library)
            idx_sb = cpool.tile([P, totcol], mybir.dt.int16)
            nc.sync.dma_start(out=idx_sb[:], in_=t_gidx[:])
            nc.gpsimd.load_library(library_config.mlp)
            s_sb = cpool.tile([P, NBINS, SW], f16)
            nc.sync.dma_start(out=s_sb[:], in_=t_s[:])
            w2_sb = cpool.tile([P, NPAIR * U], f16)
            nc.sync.dma_start(out=w2_sb[:], in_=t_w2[:])
            gk_sb = cpool.tile([U, 3 * U], f16)
            nc.sync.dma_start(out=gk_sb[:], in_=t_gk[:])
            grk_sb = cpool.tile([U, 3 * U], f16)
            nc.sync.dma_start(out=grk_sb[:], in_=t_grk[:])
            gbzr_sb = cpool.tile([U, 2], f32)
            nc.sync.dma_start(out=gbzr_sb[:], in_=t_gbzr[:])
            gbh0_sb = cpool.tile([U, 1], f32)
            nc.sync.dma_start(out=gbh0_sb[:], in_=t_gbh0[:])
            gbh1_sb = cpool.tile([U, 1], f32)
            nc.sync.dma_start(out=gbh1_sb[:], in_=t_gbh1[:])
            ident = cpool.tile([U, U], f16)
            nc.sync.dma_start(out=ident[:], in_=t_ident[:])

            hT16 = cpool.tile([U, SLOTS], f16, name="hT16_0", tag="hT16a")
            nc.sync.dma_start(out=hT16[:], in_=t_h0t16[:])
            nb0_sb = cpool.tile([P, NBINS, U], f16)
            nc.sync.dma_start(out=nb0_sb[:], in_=t_nb0[:])

            # Per-step AllGather landing buffers (the sim enforces a single
            # writer per Shared buffer).  The gather preps for step s+1 are
            # issued during step s (desc-gen off the critical path);
            # read-vs-write ordering with the collective is enforced
            # manually via the trigger's sync dep (Tile's deferred-dep
            # machinery can't express a prep issued before its source's
            # writer).
            cc_outs = [
                dpool.tile([NPAD, U], f32, name=f"cc_out{s}",
                           tag=f"cc_out{s}", addr_space="Shared")
                for s in range(STEPS - 1)
            ]
            gsems = [nc.alloc_semaphore(f"gsem{i}")
                     for i in range(NPIECE * STEPS)]
            for s_ in gsems:
                nc.gpsimd.sem_clear(s_)

            def issue_preps(step, gsrc, trigger_each=False):
                """SWDGE descriptor preps for step's gather (NPIECE pieces,
                so stage-1 on early pieces overlaps later pieces' DMA).
                Descriptors are generated now (gpsimd); the DMA fires at
                trigger_dma time, reading gsrc as it is *then*.  Queues
                1..3 (never 0): Tile's DMASW pre-bump IncSwdgeSem rides
                queue 0, and a doorbell on a queue holding untriggered
                prep descriptors would fire them early.  Data completion
                is user-managed via gsems (+16 on DMA done)."""
                q = 1 + step % 3
                nbf = wpool.tile([P, NBINS, U], f32, tag="nbf")
                for pi, (klo, khi, ilo, ihi) in enumerate(pieces):
                    nc.gpsimd.dma_gather(
                        out_ap=nbf[:, klo:khi, :], in_ap=gsrc,
                        idxs_ap=idx_sb[:, ilo:ihi],
                        num_idxs=(khi - klo) * P,
                        num_idxs_reg=(khi - klo) * P,
                        elem_size=U, single_packet=False,
                        prepare_only=True, sem=gsems[NPIECE * step + pi],
                        queue_num=q,
                    )
                    if trigger_each:
                        # step 0: source is ready, so fire each piece as
                        # soon as its descriptors land — piece 0's DMA
                        # overlaps pieces 1-3's desc-gen
                        nc.gpsimd.trigger_dma(count=None, queue_num=q)
                return nbf

            nbfs = [None] * STEPS
            cc_prev = None

            for step in range(STEPS):
                # Fire this step's gather pieces (step 0 has none: its nb
                # comes pre-gathered from the host).  The sync dep on the
                # previous AllGather makes the source ordering visible.
                # True DMA completion is gated by the per-piece CAST gsem
                # waits below.
                if step > 0:
                    trig = nc.gpsimd.trigger_dma(
                        count=None, queue_num=1 + step % 3)
                    deps = bass.InstructionNameOrderedSet()
                    for op in cc_prev:
                        deps.add(op.ins.name)
                    trig.ins.add_sync_dependencies_from(deps)
                # Generate next step's descriptors during this step's work.
                if step < STEPS - 1:
                    nbfs[step + 1] = issue_preps(step + 1, cc_outs[step][:])
                nbf = nbfs[step]

                # u is pair-major on 128 partitions: partitions 0-63 hold
                # channels 2q (plus 16), 64-127 hold channels 2q+1 (channel
                # 16 duplicated so pair 8's upper half is defined; its w2p
                # rows are zero).
                u = wpool.tile([P, NBINS, NPAIR * WBIN], f16, tag="u", bufs=1)
                hT16_new = cpool.tile([U, SLOTS], f16, name="hT16_new",
                                      tag=f"hT16b{step % 2}")
                aggT = wpool.tile([U, SLOTS], f16, tag="aggT", bufs=2)
                pack = wpool.tile([P, NPC // P, U], f32, tag="pack")
                nb = (nb0_sb if step == 0
                      else wpool.tile([P, NBINS, U], f16, tag="nb"))
                cc_in = (dpool.tile([NPC, U], f32, name=f"cc_in{step}",
                                    tag=f"cc_in{step}")
                         if step < STEPS - 1 else None)

                def cast_piece(pi):
                    klo, khi = pieces[pi][:2]
                    cast = nc.vector.tensor_copy(
                        out=nb[:, klo:khi, :], in_=nbf[:, klo:khi, :]
                    )._wait_ge(gsems[NPIECE * step + pi], 16)
                    if cc_prev is not None:
                        cdep = bass.InstructionNameOrderedSet()
                        for op in cc_prev:
                            cdep.add(op.ins.name)
                        cast.ins.add_sync_dependencies_from(cdep)

                def stage1(b):
                    # The PSUM->SBUF drain is split across Scalar (even
                    # channels -> partitions 0-63) and Vector (odd ->
                    # 64-127) so it keeps up with the matmul cadence.
                    ps_u = pp.tile([U, SW], f32, tag="ps_a", bufs=3)
                    nc.tensor.matmul(
                        out=ps_u[:],
                        lhsT=nb[:, b, :],
                        rhs=s_sb[:, b, :],
                        start=True, stop=True,
                    )
                    psv = ps_u[:].rearrange("j (f w) -> j f w", w=WBIN)
                    uv = u[:].rearrange("p b (q w) -> p b q w", w=WBIN)
                    nc.scalar.copy(out=uv[0:U, b, :, :], in_=psv[:, 0::2, :])
                    nc.vector.tensor_copy(out=uv[U:P, b, 0:NPAIR - 1, :],
                                          in_=psv[:, 1::2, :])
                    nc.vector.tensor_copy(out=uv[U:P, b, NPAIR - 1, :],
                                          in_=psv[:, FCH - 1, :])

                def stage2(lo, wdt):
                    """agg^T columns [lo, lo+wdt): PSUM matmuls + DVE copy."""
                    sl = slice(lo, lo + wdt)
                    blo, bhi = lo // WBIN, (lo + wdt) // WBIN
                    ps_agg = pp.tile([U, NH0], f32, tag="agg", bufs=1)
                    for q in range(NPAIR):
                        nc.tensor.matmul(
                            out=ps_agg[:, :wdt],
                            lhsT=w2_sb[:, q * U:(q + 1) * U],
                            rhs=u[:, blo:bhi, q * WBIN:(q + 1) * WBIN],
                            start=(q == 0), stop=(q == NPAIR - 1),
                        )
                    nc.vector.tensor_copy(out=aggT[:, sl], in_=ps_agg[:, :wdt])

                def gru(lo, wdt):
                    sl = slice(lo, lo + wdt)
                    ps_z = pp.tile([U, NH0], f32, tag="gru_ps", bufs=4)
                    nc.tensor.matmul(out=ps_z[:, :wdt],
                                     lhsT=gk_sb[:, 0:U],
                                     rhs=aggT[:, sl],
                                     start=True, stop=False)
                    nc.tensor.matmul(out=ps_z[:, :wdt],
                                     lhsT=grk_sb[:, 0:U],
                                     rhs=hT16[:, sl],
                                     start=False, stop=True)
                    ps_r = pp.tile([U, NH0], f32, tag="gru_ps", bufs=4)
                    nc.tensor.matmul(out=ps_r[:, :wdt],
                                     lhsT=gk_sb[:, U:2 * U],
                                     rhs=aggT[:, sl],
                                     start=True, stop=False)
                    nc.tensor.matmul(out=ps_r[:, :wdt],
                                     lhsT=grk_sb[:, U:2 * U],
                                     rhs=hT16[:, sl],
                                     start=False, stop=True)
                    ps_xh = pp.tile([U, NH0], f32, tag="gru_ps", bufs=4)
                    nc.tensor.matmul(out=ps_xh[:, :wdt],
                                     lhsT=gk_sb[:, 2 * U:3 * U],
                                     rhs=aggT[:, sl],
                                     start=True, stop=True)
                    ps_ih = pp.tile([U, NH0], f32, tag="gru_ps", bufs=4)
                    nc.tensor.matmul(out=ps_ih[:, :wdt],
                                     lhsT=grk_sb[:, 2 * U:3 * U],
                                     rhs=hT16[:, sl],
                                     start=True, stop=True)

                    z_sb = wpool.tile([U, NH0], f32, tag="z")
                    nc.scalar.activation(out=z_sb[:, :wdt], in_=ps_z[:, :wdt],
                                         func=AF.Sigmoid, bias=gbzr_sb[:, 0:1])
                    r_sb = wpool.tile([U, NH0], f32, tag="r")
                    nc.scalar.activation(out=r_sb[:, :wdt], in_=ps_r[:, :wdt],
                                         func=AF.Sigmoid, bias=gbzr_sb[:, 1:2])
                    t1 = wpool.tile([U, NH0], f32, tag="t1")
                    nc.vector.tensor_scalar_add(out=t1[:, :wdt],
                                                in0=ps_ih[:, :wdt],
                                                scalar1=gbh1_sb[:, 0:1])
                    nc.vector.tensor_mul(out=t1[:, :wdt], in0=r_sb[:, :wdt],
                                         in1=t1[:, :wdt])
                    nc.vector.tensor_add(out=t1[:, :wdt], in0=t1[:, :wdt],
                                         in1=ps_xh[:, :wdt])
                    hh = wpool.tile([U, NH0], f32, tag="hh")
                    nc.scalar.activation(out=hh[:, :wdt], in_=t1[:, :wdt],
                                         func=AF.Tanh, bias=gbh0_sb[:, 0:1])
                    d = wpool.tile([U, NH0], f32, tag="d")
                    nc.vector.tensor_sub(out=d[:, :wdt], in0=hT16[:, sl],
                                         in1=hh[:, :wdt])
                    nc.vector.tensor_mul(out=d[:, :wdt], in0=z_sb[:, :wdt],
                                         in1=d[:, :wdt])
                    nc.vector.tensor_add(out=hT16_new[:, sl],
                                         in0=hh[:, :wdt], in1=d[:, :wdt])

                def pack_blocks(blo, bhi):
                    """PE-transpose hT16_new 128-col blocks [blo,bhi) into
                    the node-major pack tile (upcast to f32 in the copy),
                    then DMA the block range into cc_in."""
                    for b in range(blo, bhi):
                        ps_tr = pp.tile([P, U], f16, tag="ps_a", bufs=3)
                        nc.tensor.transpose(
                            out=ps_tr[:],
                            in_=hT16_new[:, b * P:(b + 1) * P],
                            identity=ident[:])
                        nc.scalar.copy(out=pack[:, b, :], in_=ps_tr[:])
                    if cc_in is not None:
                        nc.sync.dma_start(
                            out=cc_in[blo * P:bhi * P, :].rearrange(
                                "(b p) i -> p b i", p=P),
                            in_=pack[:, blo:bhi, :])

                # ---------------- stage 1 / stage 2 / GRU / pack interleave.
                # PE executes in order; gather pieces drip in, so emit half-0
                # stage-2 right after piece 1 (bins 0..23), GRU(0)+pack(0)
                # after piece 2, and only the half-1 tail after the last
                # piece.
                for pi in range(NPIECE):
                    if step > 0:
                        cast_piece(pi)
                    for b in range(*pieces[pi][:2]):
                        stage1(b)
                    if pi == 1:
                        stage2(0, NH0)
                    elif pi == 2:
                        gru(0, NH0)
                # piece-3 stage-1 was just emitted: it fills PE while
                # gru(0)'s ACT/DVE chain runs.  The half-1 tail is split
                # into 128-col quarters so gru(1a)'s chain overlaps
                # stage-2(1b)'s matmuls.
                stage2(NH0, P)
                gru(NH0, P)
                pack_blocks(0, NH0 // P)
                stage2(NH0 + P, P)
                gru(NH0 + P, P)
                pack_blocks(NH0 // P, NPC // P)

                hT16 = hT16_new

                # ---------------- exchange (not needed after last step)
                if step < STEPS - 1:
                    cc_prev = [nc.gpsimd.collective_compute(
                        "AllGather",
                        mybir.AluOpType.bypass,
                        replica_groups=[list(range(NCORES))],
                        ins=[cc_in[:].opt()],
                        outs=[cc_outs[step][:].opt()],
                    )]

            nc.sync.dma_start(out=t_out[:], in_=hT16[:])

    nc.compile()
    return nc


# ----------------------------------------------------------------- driver
def kernel(node_features, edge_features, pair_indices, edge_kernel, edge_bias,
           gru_kernel, gru_recurrent_kernel, gru_bias):
    prep = _preprocess(node_features, edge_features, pair_indices)
    wts = _prep_weights(edge_kernel, edge_bias, gru_kernel,
                        gru_recurrent_kernel, gru_bias)

    if "nc" not in _cache:
        _cache["nc"] = _build_program()
    nc = _cache["nc"]

    in_maps = []
    for c in range(NCORES):
        in_maps.append({
            "s_mat": prep["S"][c],
            "gidx16": prep["gidx16"][c],
            "ident": np.eye(U, dtype=np.float16),
            "h0t16": prep["h0t16"][c],
            "nb0": prep["nb0"][c],
            "w2": wts["w2"],
            "gk": wts["gk"],
            "grk": wts["grk"],
            "gbzr": wts["gbzr"],
            "gbh0": wts["gbh0"],
            "gbh1": wts["gbh1"],
        })

    from concourse.bass_utils import run_bass_kernel_spmd
    res = run_bass_kernel_spmd(nc, in_maps, core_ids=list(range(NCORES)))
    outs = res.results

    h_final = np.empty((N, U), dtype=np.float32)
    node_core, node_col = prep["node_core"], prep["node_col"]
    for c in range(NCORES):
        ht = outs[c]["h_out"].astype(np.float32)       # [64, SLOTS]
        sel = np.where(node_core == c)[0]
        h_final[sel] = ht[:, node_col[sel]].T
    return h_final


if __name__ == "__main__":
    sys.path.insert(0, os.path.dirname(os.path.abspath(__file__)))
    import reference

    inputs = reference.setup_inputs()
    inputs = {k: np.asarray(v) for k, v in inputs.items()}
    t0 = time.time()
    out = kernel(**inputs)
    print("kernel() wall time:", time.time() - t0)
    exp = np.asarray(reference.reference(**reference.setup_inputs()))
    err = np.abs(out - exp).max() / (np.abs(exp).max() + 1e-30)
    print("Relative error:", err)
